# revision 23
# baseline (speedup 1.0000x reference)
"""NTM addressing head (nn_HeadBase) Trainium2 Bass kernel.

Full-input contract: kernel(**inputs) takes the unsharded [256, ...] arrays,
shards batch-dim across 8 NeuronCores (pure data parallel), runs one SPMD Bass
program per core, and gathers the full [256, 4096] output.

Per-core layout (B=32 batches, N=4096, M=64):
  memory[b] streamed in CB-batch chunks as [128, CB*32*64] SBUF tiles,
  n = p*32 + r (8 KB contiguous DRAM run per (partition, batch)).  The chunk
  DMA is a SWDGE (gpsimd) cast DMA f32->bf16: HBM traffic stays f32 but all
  phase-A elementwise work runs in bf16, where the DVE's 2x packed mode
  (1.86 elem/ns vs 0.93 for f32) is active.  Phase A per chunk:
  DVE: prod = mem*kb (kb prescaled by beta/knorm), halving tree
  64->32->16->8 + 8->1 f32 reduce for dot; ACT: square in place (bf16);
  DVE: same tree+reduce for ssq.
  Phase B runs in 2 groups of 16 batches, emitted after chunks 3 and 7 so
  group 0 fully overlaps phase A.  The softmax 1/denom is algebraically
  deferred onto the prev_w term (the final normalize kills any per-batch
  scale), partition-carry +-1 rotations for the circular shift are done as
  shift-matrix matmuls on the (otherwise idle) TensorEngine, per-batch
  broadcasts are read straight out of PSUM, and results accumulate in an
  SBUF tile flushed by one final DMA.
"""

import numpy as np

B_FULL, N, M = 256, 4096, 64
NCORES = 8
B = B_FULL // NCORES   # 32 batches per core
P = 128                # SBUF partitions
R = N // P             # 32 rows per partition; n = p*R + r
CB = 4                 # batches per phase-A chunk
NB = 16                # batches per phase-B group

_NC_CACHE = {}


def _build_body(nc, out_ap, ins):
    """Emit the kernel IR. ins: dict name->AP of DRAM inputs, out_ap: DRAM out."""
    from contextlib import ExitStack

    import concourse.bass as bass
    import concourse.tile as tile
    from concourse import mybir

    f32 = mybir.dt.float32
    bf16 = mybir.dt.bfloat16
    Alu = mybir.AluOpType
    Act = mybir.ActivationFunctionType
    Ax = mybir.AxisListType
    AP = bass.AP

    mem_ap = ins["memory"]   # [B, N, M]
    k_ap = ins["k"]          # [B, M]
    beta_ap = ins["beta"]    # [B, 1]
    pw_ap = ins["prev_w"]    # [B, N]
    g_ap = ins["g"]          # [B, 1]
    s_ap = ins["s"]          # [B, 3]
    gam_ap = ins["gamma"]    # [B, 1]

    def bcast_inner(ap2d, n):
        # [P, C] -> [P, C, n] with 0-stride inner dim
        return AP(ap2d.tensor, ap2d.offset, list(ap2d.ap) + [[0, n]])

    def row1(ap1d):
        # prepend a unit partition dim to a 1-d AP
        return AP(ap1d.tensor, ap1d.offset, [[0, 1]] + list(ap1d.ap))

    def colsq(ap3d):
        # [P, C, 1] -> [P, C]
        return AP(ap3d.tensor, ap3d.offset, list(ap3d.ap)[:2])

    with tile.TileContext(nc) as tc, ExitStack() as ctx:
        singles = ctx.enter_context(tc.tile_pool(name="singles", bufs=1))
        mem_pool = ctx.enter_context(tc.tile_pool(name="mem", bufs=3))
        sq_pool = ctx.enter_context(tc.tile_pool(name="sq", bufs=2))
        prod_pool = ctx.enter_context(tc.tile_pool(name="prod", bufs=1))
        scr_pool = ctx.enter_context(tc.tile_pool(name="scr", bufs=2))
        grp_pool = ctx.enter_context(tc.tile_pool(name="grp", bufs=2))
        big = ctx.enter_context(tc.tile_pool(name="big", bufs=1))
        ps = ctx.enter_context(tc.tile_pool(name="ps", bufs=2, space="PSUM"))
        ps_g = ctx.enter_context(tc.tile_pool(name="psg", bufs=2, space="PSUM"))
        ps_kb = ctx.enter_context(tc.tile_pool(name="pskb", bufs=2, space="PSUM"))

        # ---- setup: constants, per-batch scalar rows on partition 0 ----
        ones_col = singles.tile([P, 1], f32, tag="ones_col")
        nc.vector.memset(ones_col, 1.0)
        ones_row = singles.tile([1, P], f32, tag="ones_row")
        nc.vector.memset(ones_row, 1.0)
        ones_pp = singles.tile([P, P], f32, tag="ones_pp")
        nc.vector.memset(ones_pp, 1.0)

        # circular +-1 partition-shift matrices for the TensorEngine:
        # Mdn[p, i] = 1 iff i == (p+1) % 128 ; Mup[p, i] = 1 iff i == (p-1) % 128
        Mdn = singles.tile([P, P], f32, tag="Mdn")
        Mdn_b = singles.tile([P, P], f32, tag="Mdn_b")
        nc.gpsimd.affine_select(
            out=Mdn, in_=ones_pp, pattern=[[1, P]], compare_op=Alu.is_equal,
            fill=0.0, base=-1, channel_multiplier=-1,
        )
        nc.gpsimd.affine_select(
            out=Mdn_b, in_=ones_pp, pattern=[[1, P]], compare_op=Alu.is_equal,
            fill=0.0, base=P - 1, channel_multiplier=-1,
        )
        nc.vector.tensor_add(out=Mdn, in0=Mdn, in1=Mdn_b)
        Mup = singles.tile([P, P], f32, tag="Mup")
        Mup_b = singles.tile([P, P], f32, tag="Mup_b")
        nc.gpsimd.affine_select(
            out=Mup, in_=ones_pp, pattern=[[1, P]], compare_op=Alu.is_equal,
            fill=0.0, base=1, channel_multiplier=-1,
        )
        nc.gpsimd.affine_select(
            out=Mup_b, in_=ones_pp, pattern=[[1, P]], compare_op=Alu.is_equal,
            fill=0.0, base=-(P - 1), channel_multiplier=-1,
        )
        nc.vector.tensor_add(out=Mup, in0=Mup, in1=Mup_b)

        k_row = singles.tile([1, B * M], f32, tag="k_row")
        nc.sync.dma_start(out=k_row, in_=row1(k_ap.rearrange("b m -> (b m)")))
        b_row = singles.tile([1, B], f32, tag="b_row")
        nc.sync.dma_start(out=b_row, in_=row1(beta_ap.rearrange("b one -> (b one)")))
        g_row = singles.tile([1, B], f32, tag="g_row")
        nc.sync.dma_start(out=g_row, in_=row1(g_ap.rearrange("b one -> (b one)")))
        gm_row = singles.tile([1, B], f32, tag="gm_row")
        nc.sync.dma_start(out=gm_row, in_=row1(gam_ap.rearrange("b one -> (b one)")))
        s_row = singles.tile([1, 3 * B], f32, tag="s_row")
        nc.sync.dma_start(out=s_row, in_=row1(s_ap.rearrange("b i -> (b i)")))
        s_perm = s_row.rearrange("p (b i) -> p i b", i=3)
        s_v = [s_perm[:, i, :] for i in range(3)]

        # k broadcast to all partitions right away (keeps chunk 0 off the
        # knorm critical path); beta/knorm is applied in phase B via BK.
        k_row2 = scr_pool.tile([1, B * M], f32, tag="krow")
        nc.vector.tensor_copy(k_row2, k_row)
        kb = singles.tile([P, B * M], bf16, tag="kb")
        for j in range(0, B * M, 512):
            kb_ps = ps_kb.tile([P, 512], f32, tag="kbmm")
            nc.tensor.matmul(
                kb_ps, ones_row, k_row2[:, j : j + 512], start=True, stop=True,
            )
            nc.scalar.copy(out=kb[:, j : j + 512], in_=kb_ps)

        # knorm; bk = beta / knorm
        ksq_row = scr_pool.tile([1, B * M], f32, tag="krow")
        nc.vector.tensor_mul(ksq_row, k_row, k_row)
        ks_row = singles.tile([1, B], f32, tag="ks_row")
        nc.vector.tensor_reduce(
            out=ks_row, in_=ksq_row.rearrange("p (b m) -> p b m", m=M),
            axis=Ax.X, op=Alu.add,
        )
        kn_row = singles.tile([1, B], f32, tag="kn_row")
        nc.scalar.activation(out=kn_row, in_=ks_row, func=Act.Sqrt)
        rk_row = singles.tile([1, B], f32, tag="rk_row")
        nc.vector.reciprocal(out=rk_row, in_=kn_row)
        bk_row = singles.tile([1, B], f32, tag="bk_row")
        nc.vector.tensor_mul(bk_row, b_row, rk_row)

        # omg = 1 - g
        omg_row = singles.tile([1, B], f32, tag="omg_row")
        nc.vector.tensor_scalar(
            out=omg_row, in0=g_row, scalar1=-1.0, scalar2=1.0,
            op0=Alu.mult, op1=Alu.add,
        )

        # broadcast round: [g, omg, s0, s1, s2, gamma, bk] -> [P, 7*B]
        NSC = 7
        asm1 = singles.tile([1, NSC * B], f32, tag="asm1")
        for i, src in enumerate(
            [g_row, omg_row, s_v[0], s_v[1], s_v[2], gm_row, bk_row]
        ):
            nc.vector.tensor_copy(asm1[:, i * B : (i + 1) * B], src)
        bc1_ps = ps.tile([P, NSC * B], f32, tag="mm")
        nc.tensor.matmul(bc1_ps, ones_row, asm1, start=True, stop=True)
        BC1 = singles.tile([P, NSC * B], f32, tag="BC1")
        nc.scalar.copy(out=BC1, in_=bc1_ps)
        G_B = BC1[:, 0 * B : 1 * B]
        OMG = BC1[:, 1 * B : 2 * B]
        S0 = BC1[:, 2 * B : 3 * B]
        S1 = BC1[:, 3 * B : 4 * B]
        S2 = BC1[:, 4 * B : 5 * B]
        GAM = BC1[:, 5 * B : 6 * B]
        BK = BC1[:, 6 * B : 7 * B]

        # prev_w big tile ((1-g) rides in the deferred-D coefficient c2)
        pw = big.tile([P, B * R], f32, tag="pw")
        nc.sync.dma_start(
            out=pw.rearrange("p (b r) -> p b r", r=R),
            in_=pw_ap.rearrange("b (p r) -> p b r", r=R),
        )

        # ---- phase A chunks + phase B groups ----
        dot = big.tile([P, B * R], f32, tag="dot")
        ssq = big.tile([P, B * R], f32, tag="ssq")
        out_sb = big.tile([P, B * R], f32, tag="out_sb")
        FCH = CB * R * M

        def phase_a_chunk(c):
            b0 = c * CB
            mt = mem_pool.tile([P, FCH], bf16, tag="mt")
            nc.gpsimd.dma_start(
                out=mt.rearrange("p (b f) -> p b f", b=CB),
                in_=mem_ap[b0 : b0 + CB].rearrange(
                    "b (p r) m -> p b (r m)", p=P
                ),
            )
            mt4 = mt.rearrange("p (b r m) -> p b r m", b=CB, m=M)
            mt3 = mt.rearrange("p (g m) -> p g m", m=M)
            prod = prod_pool.tile([P, FCH], bf16, tag="prod")
            pr3 = prod.rearrange("p (g m) -> p g m", m=M)
            kbc = kb[:, b0 * M : (b0 + CB) * M]  # [P, CB*M] bf16
            kb4 = AP(
                kbc.tensor, kbc.offset,
                [kbc.ap[0], [M, CB], [0, R], [1, M]],
            )
            # dot path on DVE: prod = mem*kb, tree 64->8, reduce 8->1 (f32)
            nc.vector.tensor_tensor(
                out=prod.rearrange("p (b r m) -> p b r m", b=CB, m=M),
                in0=mt4, in1=kb4, op=Alu.mult,
            )
            for w in (32, 16, 8):
                nc.vector.tensor_add(
                    out=pr3[:, :, 0:w], in0=pr3[:, :, 0:w],
                    in1=pr3[:, :, w : 2 * w],
                )
            nc.vector.tensor_reduce(
                out=dot[:, b0 * R : (b0 + CB) * R],
                in_=pr3[:, :, 0:8], axis=Ax.X, op=Alu.add,
            )
            # ssq path: square on ACT (bf16, runs parallel to the mult),
            # tree + reduce on DVE
            sq = sq_pool.tile([P, FCH], bf16, tag="sq")
            nc.scalar.square(out=sq, in_=mt)
            sq3 = sq.rearrange("p (g m) -> p g m", m=M)
            for w in (32, 16, 8):
                nc.vector.tensor_add(
                    out=sq3[:, :, 0:w], in0=sq3[:, :, 0:w],
                    in1=sq3[:, :, w : 2 * w],
                )
            nc.vector.tensor_reduce(
                out=ssq[:, b0 * R : (b0 + CB) * R],
                in_=sq3[:, :, 0:8], axis=Ax.X, op=Alu.add,
            )

        def phase_b_group(gi):
            b0 = gi * NB
            bs = slice(b0 * R, (b0 + NB) * R)
            dz = [P, NB * R]

            def g3(t):
                return t.rearrange("p (b r) -> p b r", r=R)

            # rstd = 1/sqrt(ssq); a = (beta/knorm * dot) * rstd
            mn = grp_pool.tile(dz, f32, tag="mn")
            nc.scalar.activation(out=mn, in_=ssq[:, bs], func=Act.Sqrt)
            rstd = grp_pool.tile(dz, f32, tag="rstd")
            scr = grp_pool.tile(dz, f32, tag="scr")
            nc.vector.reciprocal_approx_accurate(out=rstd, in_=mn, scratch=scr)
            t1 = grp_pool.tile(dz, f32, tag="t1")
            nc.vector.tensor_mul(t1, dot[:, bs], rstd)
            a = grp_pool.tile(dz, f32, tag="a")
            nc.vector.tensor_mul(
                g3(a), g3(t1), bcast_inner(BK[:, b0 : b0 + NB], R)
            )
            # e = exp(a)  (unnormalized softmax)
            e = grp_pool.tile(dz, f32, tag="e")
            nc.scalar.activation(out=e, in_=a, func=Act.Exp)

            # denom D per batch; deferred normalization:
            # wg' = g*e + ((1-g)*D)*pw   (any per-batch scale cancels at the end)
            cs = grp_pool.tile([P, NB], f32, tag="cs")
            nc.vector.tensor_reduce(out=cs, in_=g3(e), axis=Ax.X, op=Alu.add)
            den_ps = ps_g.tile([1, NB], f32, tag="mmr")
            nc.tensor.matmul(den_ps, ones_col, cs, start=True, stop=True)
            c2_row = grp_pool.tile([1, NB], f32, tag="c2_row")
            nc.vector.tensor_mul(c2_row, den_ps, omg_row[:, b0 : b0 + NB])
            c2_ps = ps_g.tile([P, NB], f32, tag="mmb")
            nc.tensor.matmul(c2_ps, ones_row, c2_row, start=True, stop=True)

            wg = grp_pool.tile(dz, f32, tag="wg")
            nc.vector.tensor_mul(g3(wg), g3(e), bcast_inner(G_B[:, b0 : b0 + NB], R))
            t2 = grp_pool.tile(dz, f32, tag="t2")
            nc.vector.tensor_mul(
                g3(t2), g3(pw[:, bs]), bcast_inner(c2_ps, R)
            )
            nc.vector.tensor_add(out=wg, in0=wg, in1=t2)

            # circular 3-tap shift: ws[n] = s1*wg[n] + s0*wg[n-1] + s2*wg[n+1]
            ws = grp_pool.tile(dz, f32, tag="ws")
            ta = grp_pool.tile(dz, f32, tag="ta")
            tb = grp_pool.tile(dz, f32, tag="tb")
            wg3, ws3, ta3, tb3 = g3(wg), g3(ws), g3(ta), g3(tb)
            sl = slice(b0, b0 + NB)
            nc.vector.tensor_mul(ta3, wg3, bcast_inner(S0[:, sl], R))
            nc.vector.tensor_mul(tb3, wg3, bcast_inner(S2[:, sl], R))
            nc.vector.tensor_mul(ws3, wg3, bcast_inner(S1[:, sl], R))
            nc.vector.tensor_add(
                out=ws3[:, :, 1:R], in0=ws3[:, :, 1:R], in1=ta3[:, :, 0 : R - 1]
            )
            nc.vector.tensor_add(
                out=ws3[:, :, 0 : R - 1], in0=ws3[:, :, 0 : R - 1],
                in1=tb3[:, :, 1:R],
            )
            # partition carries via TensorEngine shift matmuls
            dn_ps = ps_g.tile([P, NB], f32, tag="mmb")
            nc.tensor.matmul(
                dn_ps, Mdn, colsq(ta3[:, :, R - 1 : R]), start=True, stop=True
            )
            up_ps = ps_g.tile([P, NB], f32, tag="mmb")
            nc.tensor.matmul(
                up_ps, Mup, colsq(tb3[:, :, 0:1]), start=True, stop=True
            )
            nc.vector.tensor_add(
                out=ws3[:, :, 0:1], in0=ws3[:, :, 0:1], in1=bcast_inner(dn_ps, 1)
            )
            nc.vector.tensor_add(
                out=ws3[:, :, R - 1 : R], in0=ws3[:, :, R - 1 : R],
                in1=bcast_inner(up_ps, 1),
            )

            # w_pow = ws ** gamma = exp(gamma * ln(ws))
            nc.scalar.activation(out=ws, in_=ws, func=Act.Ln)
            nc.vector.tensor_mul(ws3, ws3, bcast_inner(GAM[:, sl], R))
            nc.scalar.activation(out=ws, in_=ws, func=Act.Exp)

            # normalize into the staging tile: out = w_pow / sum
            cs2 = grp_pool.tile([P, NB], f32, tag="cs2")
            nc.vector.tensor_reduce(out=cs2, in_=ws3, axis=Ax.X, op=Alu.add)
            d2_ps = ps_g.tile([1, NB], f32, tag="mmr")
            nc.tensor.matmul(d2_ps, ones_col, cs2, start=True, stop=True)
            rd2_row = grp_pool.tile([1, NB], f32, tag="rd2_row")
            nc.vector.reciprocal(out=rd2_row, in_=d2_ps)
            rd2_ps = ps_g.tile([P, NB], f32, tag="mmb")
            nc.tensor.matmul(rd2_ps, ones_row, rd2_row, start=True, stop=True)
            ob3 = out_sb.rearrange("p (b r) -> p b r", r=R)
            nc.vector.tensor_mul(
                ob3[:, b0 : b0 + NB, :], ws3, bcast_inner(rd2_ps, R)
            )

        ngroup = B // NB
        chunks_per_group = (B // CB) // ngroup
        for gi in range(ngroup):
            for cc in range(chunks_per_group):
                phase_a_chunk(gi * chunks_per_group + cc)
            phase_b_group(gi)

        nc.sync.dma_start(
            out=out_ap.rearrange("b (p r) -> p b r", r=R),
            in_=out_sb.rearrange("p (b r) -> p b r", r=R),
        )


def _get_nc():
    if "nc" in _NC_CACHE:
        return _NC_CACHE["nc"]
    from concourse import bacc, mybir

    f32 = mybir.dt.float32
    nc = bacc.Bacc("TRN2", debug=False, num_devices=NCORES)
    ins = {
        "memory": nc.dram_tensor("memory", [B, N, M], f32, kind="ExternalInput").ap(),
        "k": nc.dram_tensor("k", [B, M], f32, kind="ExternalInput").ap(),
        "beta": nc.dram_tensor("beta", [B, 1], f32, kind="ExternalInput").ap(),
        "prev_w": nc.dram_tensor("prev_w", [B, N], f32, kind="ExternalInput").ap(),
        "g": nc.dram_tensor("g", [B, 1], f32, kind="ExternalInput").ap(),
        "s": nc.dram_tensor("s", [B, 3], f32, kind="ExternalInput").ap(),
        "gamma": nc.dram_tensor("gamma", [B, 1], f32, kind="ExternalInput").ap(),
    }
    out_ap = nc.dram_tensor("out", [B, N], f32, kind="ExternalOutput").ap()
    _build_body(nc, out_ap, ins)
    nc.finalize()
    _NC_CACHE["nc"] = nc
    return nc


def _shard_inputs(inputs):
    arrs = {
        name: np.ascontiguousarray(np.asarray(inputs[name], dtype=np.float32))
        for name in ("memory", "k", "beta", "prev_w", "g", "s", "gamma")
    }
    in_maps = []
    for c in range(NCORES):
        sl = slice(c * B, (c + 1) * B)
        in_maps.append({name: np.ascontiguousarray(a[sl]) for name, a in arrs.items()})
    return in_maps


def run(inputs, trace=False):
    from concourse.bass_utils import run_bass_kernel_spmd

    nc = _get_nc()
    in_maps = _shard_inputs(inputs)
    res = run_bass_kernel_spmd(
        nc, in_maps, core_ids=list(range(NCORES)), trace=trace,
        **({"trace_cores": [0]} if trace else {}),
    )
    out = np.concatenate([r["out"] for r in res.results], axis=0)
    return out, res


def kernel(**inputs):
    out, _ = run(inputs, trace=False)
    return out


# revision 28
# speedup vs baseline: 1.1780x; 1.1780x over previous
"""NTM addressing head (nn_HeadBase) Trainium2 Bass kernel.

Full-input contract: kernel(**inputs) takes the unsharded [256, ...] arrays,
shards batch-dim across 8 NeuronCores (pure data parallel), runs one SPMD Bass
program per core, and gathers the full [256, 4096] output.

Per-core layout (B=32 batches, N=4096, M=64):
  memory[b] streamed in CB-batch chunks as [128, CB*32*64] SBUF tiles,
  n = p*32 + r (8 KB contiguous DRAM run per (partition, batch)).  The chunk
  DMA is a SWDGE (gpsimd) cast DMA f32->bf16: HBM traffic stays f32 but all
  phase-A elementwise work runs in bf16, where the DVE's 2x packed mode
  (1.86 elem/ns vs 0.93 for f32) is active.  Phase A per chunk:
  DVE: prod = mem*kb (kb prescaled by beta/knorm), halving tree
  64->32->16->8 + 8->1 f32 reduce for dot; ACT: square in place (bf16);
  DVE: same tree+reduce for ssq.
  Phase B runs in 2 groups of 16 batches, emitted after chunks 3 and 7 so
  group 0 fully overlaps phase A.  The softmax 1/denom is algebraically
  deferred onto the prev_w term (the final normalize kills any per-batch
  scale), partition-carry +-1 rotations for the circular shift are done as
  shift-matrix matmuls on the (otherwise idle) TensorEngine, per-batch
  broadcasts are read straight out of PSUM, and results accumulate in an
  SBUF tile flushed by one final DMA.
"""

import numpy as np

B_FULL, N, M = 256, 4096, 64
NCORES = 8
B = B_FULL // NCORES   # 32 batches per core
P = 128                # SBUF partitions
R = N // P             # 32 rows per partition; n = p*R + r
CB = 4                 # batches per phase-A chunk
NB = 16                # batches per phase-B group

_NC_CACHE = {}


def _build_body(nc, out_ap, ins):
    """Emit the kernel IR. ins: dict name->AP of DRAM inputs, out_ap: DRAM out."""
    from contextlib import ExitStack

    import concourse.bass as bass
    import concourse.tile as tile
    from concourse import mybir

    f32 = mybir.dt.float32
    bf16 = mybir.dt.bfloat16
    Alu = mybir.AluOpType
    Act = mybir.ActivationFunctionType
    Ax = mybir.AxisListType
    AP = bass.AP

    mem_ap = ins["memory"]   # [B, N, M]
    k_ap = ins["k"]          # [B, M]
    beta_ap = ins["beta"]    # [B, 1]
    pw_ap = ins["prev_w"]    # [B, N]
    g_ap = ins["g"]          # [B, 1]
    s_ap = ins["s"]          # [B, 3]
    gam_ap = ins["gamma"]    # [B, 1]

    def bcast_inner(ap2d, n):
        # [P, C] -> [P, C, n] with 0-stride inner dim
        return AP(ap2d.tensor, ap2d.offset, list(ap2d.ap) + [[0, n]])

    def row1(ap1d):
        # prepend a unit partition dim to a 1-d AP
        return AP(ap1d.tensor, ap1d.offset, [[0, 1]] + list(ap1d.ap))

    def colsq(ap3d):
        # [P, C, 1] -> [P, C]
        return AP(ap3d.tensor, ap3d.offset, list(ap3d.ap)[:2])

    with tile.TileContext(nc) as tc, ExitStack() as ctx:
        singles = ctx.enter_context(tc.tile_pool(name="singles", bufs=1))
        mem_pool = ctx.enter_context(tc.tile_pool(name="mem", bufs=3))
        prod_pool = ctx.enter_context(tc.tile_pool(name="prod", bufs=1))
        scr_pool = ctx.enter_context(tc.tile_pool(name="scr", bufs=2))
        grp_pool = ctx.enter_context(tc.tile_pool(name="grp", bufs=2))
        big = ctx.enter_context(tc.tile_pool(name="big", bufs=1))
        ps = ctx.enter_context(tc.tile_pool(name="ps", bufs=2, space="PSUM"))
        ps_g = ctx.enter_context(tc.tile_pool(name="psg", bufs=2, space="PSUM"))
        ps_kb = ctx.enter_context(tc.tile_pool(name="pskb", bufs=2, space="PSUM"))

        # ---- prefetch: first chunk DMAs go on the gpsimd queue before any
        # other gpsimd work so the memory stream starts immediately ----
        FCH = CB * R * M
        mts = {}

        def phase_a_dma(c):
            b0 = c * CB
            mt = mem_pool.tile([P, FCH], bf16, tag="mt")
            nc.gpsimd.dma_start(
                out=mt.rearrange("p (b f) -> p b f", b=CB),
                in_=mem_ap[b0 : b0 + CB].rearrange(
                    "b (p r) m -> p b (r m)", p=P
                ),
            )
            mts[c] = mt

        for c in range(3):
            phase_a_dma(c)

        # ---- setup: constants, per-batch scalar rows on partition 0 ----
        ones_col = singles.tile([P, 1], f32, tag="ones_col")
        nc.vector.memset(ones_col, 1.0)
        ones_row = singles.tile([1, P], f32, tag="ones_row")
        nc.vector.memset(ones_row, 1.0)
        ones_pp = singles.tile([P, P], f32, tag="ones_pp")
        nc.vector.memset(ones_pp, 1.0)

        # circular +-1 partition-shift matrices for the TensorEngine:
        # Mdn[p, i] = 1 iff i == (p+1) % 128 ; Mup[p, i] = 1 iff i == (p-1) % 128
        Mdn = singles.tile([P, P], f32, tag="Mdn")
        Mdn_b = singles.tile([P, P], f32, tag="Mdn_b")
        nc.gpsimd.affine_select(
            out=Mdn, in_=ones_pp, pattern=[[1, P]], compare_op=Alu.is_equal,
            fill=0.0, base=-1, channel_multiplier=-1,
        )
        nc.gpsimd.affine_select(
            out=Mdn_b, in_=ones_pp, pattern=[[1, P]], compare_op=Alu.is_equal,
            fill=0.0, base=P - 1, channel_multiplier=-1,
        )
        nc.vector.tensor_add(out=Mdn, in0=Mdn, in1=Mdn_b)
        Mup = singles.tile([P, P], f32, tag="Mup")
        Mup_b = singles.tile([P, P], f32, tag="Mup_b")
        nc.gpsimd.affine_select(
            out=Mup, in_=ones_pp, pattern=[[1, P]], compare_op=Alu.is_equal,
            fill=0.0, base=1, channel_multiplier=-1,
        )
        nc.gpsimd.affine_select(
            out=Mup_b, in_=ones_pp, pattern=[[1, P]], compare_op=Alu.is_equal,
            fill=0.0, base=-(P - 1), channel_multiplier=-1,
        )
        nc.vector.tensor_add(out=Mup, in0=Mup, in1=Mup_b)

        k_row = singles.tile([1, B * M], f32, tag="k_row")
        nc.sync.dma_start(out=k_row, in_=row1(k_ap.rearrange("b m -> (b m)")))
        b_row = singles.tile([1, B], f32, tag="b_row")
        nc.sync.dma_start(out=b_row, in_=row1(beta_ap.rearrange("b one -> (b one)")))
        g_row = singles.tile([1, B], f32, tag="g_row")
        nc.sync.dma_start(out=g_row, in_=row1(g_ap.rearrange("b one -> (b one)")))
        gm_row = singles.tile([1, B], f32, tag="gm_row")
        nc.sync.dma_start(out=gm_row, in_=row1(gam_ap.rearrange("b one -> (b one)")))
        s_row = singles.tile([1, 3 * B], f32, tag="s_row")
        nc.sync.dma_start(out=s_row, in_=row1(s_ap.rearrange("b i -> (b i)")))
        s_perm = s_row.rearrange("p (b i) -> p i b", i=3)
        s_v = [s_perm[:, i, :] for i in range(3)]

        # k broadcast to all partitions right away (keeps chunk 0 off the
        # knorm critical path); beta/knorm is applied in phase B via BK.
        # bf16 matmul: single PE pass (fp32 matmuls take LOW+HIGH passes).
        ones_row_bf = singles.tile([1, P], bf16, tag="ones_row_bf")
        nc.vector.memset(ones_row_bf, 1.0)
        k_row2 = scr_pool.tile([1, B * M], bf16, tag="krowbf")
        nc.vector.tensor_copy(k_row2, k_row)
        kb = singles.tile([P, B * M], bf16, tag="kb")
        for j in range(0, B * M, 512):
            kb_ps = ps_kb.tile([P, 512], f32, tag="kbmm")
            nc.tensor.matmul(
                kb_ps, ones_row_bf, k_row2[:, j : j + 512], start=True, stop=True,
            )
            nc.scalar.copy(out=kb[:, j : j + 512], in_=kb_ps)

        # knorm; bk = beta / knorm
        ksq_row = scr_pool.tile([1, B * M], f32, tag="krow")
        nc.vector.tensor_mul(ksq_row, k_row, k_row)
        ks_row = singles.tile([1, B], f32, tag="ks_row")
        nc.vector.tensor_reduce(
            out=ks_row, in_=ksq_row.rearrange("p (b m) -> p b m", m=M),
            axis=Ax.X, op=Alu.add,
        )
        kn_row = singles.tile([1, B], f32, tag="kn_row")
        nc.scalar.activation(out=kn_row, in_=ks_row, func=Act.Sqrt)
        rk_row = singles.tile([1, B], f32, tag="rk_row")
        nc.vector.reciprocal(out=rk_row, in_=kn_row)
        bk_row = singles.tile([1, B], f32, tag="bk_row")
        nc.vector.tensor_mul(bk_row, b_row, rk_row)

        # omg = 1 - g
        omg_row = singles.tile([1, B], f32, tag="omg_row")
        nc.vector.tensor_scalar(
            out=omg_row, in0=g_row, scalar1=-1.0, scalar2=1.0,
            op0=Alu.mult, op1=Alu.add,
        )

        # broadcast round: [g, omg, s0, s1, s2, gamma, bk] -> [P, 7*B]
        NSC = 7
        asm1 = singles.tile([1, NSC * B], f32, tag="asm1")
        for i, src in enumerate(
            [g_row, omg_row, s_v[0], s_v[1], s_v[2], gm_row, bk_row]
        ):
            nc.vector.tensor_copy(asm1[:, i * B : (i + 1) * B], src)
        bc1_ps = ps.tile([P, NSC * B], f32, tag="mm")
        nc.tensor.matmul(bc1_ps, ones_row, asm1, start=True, stop=True)
        BC1 = singles.tile([P, NSC * B], f32, tag="BC1")
        nc.scalar.copy(out=BC1, in_=bc1_ps)
        G_B = BC1[:, 0 * B : 1 * B]
        OMG = BC1[:, 1 * B : 2 * B]
        S0 = BC1[:, 2 * B : 3 * B]
        S1 = BC1[:, 3 * B : 4 * B]
        S2 = BC1[:, 4 * B : 5 * B]
        GAM = BC1[:, 5 * B : 6 * B]
        BK = BC1[:, 6 * B : 7 * B]

        # prev_w big tile ((1-g) rides in the deferred-D coefficient c2)
        pw = big.tile([P, B * R], f32, tag="pw")
        nc.sync.dma_start(
            out=pw.rearrange("p (b r) -> p b r", r=R),
            in_=pw_ap.rearrange("b (p r) -> p b r", r=R),
        )

        # ---- phase A chunks + phase B groups ----
        dot = big.tile([P, B * R], f32, tag="dot")
        ssq = big.tile([P, B * R], f32, tag="ssq")
        out_sb = big.tile([P, B * R], f32, tag="out_sb")

        def phase_a_chunk(c):
            b0 = c * CB
            mt = mts.pop(c)
            mt4 = mt.rearrange("p (b r m) -> p b r m", b=CB, m=M)
            mt3 = mt.rearrange("p (g m) -> p g m", m=M)
            prod = prod_pool.tile([P, FCH], bf16, tag="prod")
            pr3 = prod.rearrange("p (g m) -> p g m", m=M)
            kbc = kb[:, b0 * M : (b0 + CB) * M]  # [P, CB*M] bf16
            kb4 = AP(
                kbc.tensor, kbc.offset,
                [kbc.ap[0], [M, CB], [0, R], [1, M]],
            )
            # dot path on DVE: prod = mem*kb, tree 64->8, reduce 8->1 (f32)
            nc.vector.tensor_tensor(
                out=prod.rearrange("p (b r m) -> p b r m", b=CB, m=M),
                in0=mt4, in1=kb4, op=Alu.mult,
            )
            for w in (32, 16, 8):
                nc.vector.tensor_add(
                    out=pr3[:, :, 0:w], in0=pr3[:, :, 0:w],
                    in1=pr3[:, :, w : 2 * w],
                )
            nc.vector.tensor_reduce(
                out=dot[:, b0 * R : (b0 + CB) * R],
                in_=pr3[:, :, 0:8], axis=Ax.X, op=Alu.add,
            )
            # ssq path: square in place on ACT (serializes vs the mult's mt
            # read, which avoids SBUF port contention), tree + reduce on DVE
            nc.scalar.square(out=mt, in_=mt)
            for w in (32, 16, 8):
                nc.vector.tensor_add(
                    out=mt3[:, :, 0:w], in0=mt3[:, :, 0:w],
                    in1=mt3[:, :, w : 2 * w],
                )
            nc.vector.tensor_reduce(
                out=ssq[:, b0 * R : (b0 + CB) * R],
                in_=mt3[:, :, 0:8], axis=Ax.X, op=Alu.add,
            )
            if c + 3 < B // CB:
                phase_a_dma(c + 3)

        def phase_b_group(gi):
            b0 = gi * NB
            bs = slice(b0 * R, (b0 + NB) * R)
            dz = [P, NB * R]

            def g3(t):
                return t.rearrange("p (b r) -> p b r", r=R)

            # rstd = 1/sqrt(ssq); a = (beta/knorm * dot) * rstd
            mn = grp_pool.tile(dz, f32, tag="mn")
            nc.scalar.activation(out=mn, in_=ssq[:, bs], func=Act.Sqrt)
            rstd = grp_pool.tile(dz, f32, tag="rstd")
            scr = grp_pool.tile(dz, f32, tag="scr")
            nc.vector.reciprocal_approx_accurate(out=rstd, in_=mn, scratch=scr)
            t1 = grp_pool.tile(dz, f32, tag="t1")
            nc.vector.tensor_mul(t1, dot[:, bs], rstd)
            a = grp_pool.tile(dz, f32, tag="a")
            nc.vector.tensor_mul(
                g3(a), g3(t1), bcast_inner(BK[:, b0 : b0 + NB], R)
            )
            # e = exp(a)  (unnormalized softmax)
            e = grp_pool.tile(dz, f32, tag="e")
            nc.scalar.activation(out=e, in_=a, func=Act.Exp)

            # denom D per batch; deferred normalization:
            # wg' = g*e + ((1-g)*D)*pw   (any per-batch scale cancels at the end)
            cs = grp_pool.tile([P, NB], f32, tag="cs")
            nc.vector.tensor_reduce(out=cs, in_=g3(e), axis=Ax.X, op=Alu.add)
            den_ps = ps_g.tile([1, NB], f32, tag="mmr")
            nc.tensor.matmul(den_ps, ones_col, cs, start=True, stop=True)
            c2_row = grp_pool.tile([1, NB], f32, tag="c2_row")
            nc.vector.tensor_mul(c2_row, den_ps, omg_row[:, b0 : b0 + NB])
            c2_ps = ps_g.tile([P, NB], f32, tag="mmb")
            nc.tensor.matmul(c2_ps, ones_row, c2_row, start=True, stop=True)

            wg = grp_pool.tile(dz, f32, tag="wg")
            nc.vector.tensor_mul(g3(wg), g3(e), bcast_inner(G_B[:, b0 : b0 + NB], R))
            t2 = grp_pool.tile(dz, f32, tag="t2")
            nc.vector.tensor_mul(
                g3(t2), g3(pw[:, bs]), bcast_inner(c2_ps, R)
            )
            nc.vector.tensor_add(out=wg, in0=wg, in1=t2)

            # circular 3-tap shift: ws[n] = s1*wg[n] + s0*wg[n-1] + s2*wg[n+1]
            ws = grp_pool.tile(dz, f32, tag="ws")
            ta = grp_pool.tile(dz, f32, tag="ta")
            tb = grp_pool.tile(dz, f32, tag="tb")
            wg3, ws3, ta3, tb3 = g3(wg), g3(ws), g3(ta), g3(tb)
            sl = slice(b0, b0 + NB)
            nc.vector.tensor_mul(ta3, wg3, bcast_inner(S0[:, sl], R))
            nc.vector.tensor_mul(tb3, wg3, bcast_inner(S2[:, sl], R))
            nc.vector.tensor_mul(ws3, wg3, bcast_inner(S1[:, sl], R))
            nc.vector.tensor_add(
                out=ws3[:, :, 1:R], in0=ws3[:, :, 1:R], in1=ta3[:, :, 0 : R - 1]
            )
            nc.vector.tensor_add(
                out=ws3[:, :, 0 : R - 1], in0=ws3[:, :, 0 : R - 1],
                in1=tb3[:, :, 1:R],
            )
            # partition carries via TensorEngine shift matmuls
            dn_ps = ps_g.tile([P, NB], f32, tag="mmb")
            nc.tensor.matmul(
                dn_ps, Mdn, colsq(ta3[:, :, R - 1 : R]), start=True, stop=True
            )
            up_ps = ps_g.tile([P, NB], f32, tag="mmb")
            nc.tensor.matmul(
                up_ps, Mup, colsq(tb3[:, :, 0:1]), start=True, stop=True
            )
            nc.vector.tensor_add(
                out=ws3[:, :, 0:1], in0=ws3[:, :, 0:1], in1=bcast_inner(dn_ps, 1)
            )
            nc.vector.tensor_add(
                out=ws3[:, :, R - 1 : R], in0=ws3[:, :, R - 1 : R],
                in1=bcast_inner(up_ps, 1),
            )

            # w_pow = ws ** gamma = exp(gamma * ln(ws))
            nc.scalar.activation(out=ws, in_=ws, func=Act.Ln)
            nc.vector.tensor_mul(ws3, ws3, bcast_inner(GAM[:, sl], R))
            nc.scalar.activation(out=ws, in_=ws, func=Act.Exp)

            # normalize into the staging tile: out = w_pow / sum
            cs2 = grp_pool.tile([P, NB], f32, tag="cs2")
            nc.vector.tensor_reduce(out=cs2, in_=ws3, axis=Ax.X, op=Alu.add)
            d2_ps = ps_g.tile([1, NB], f32, tag="mmr")
            nc.tensor.matmul(d2_ps, ones_col, cs2, start=True, stop=True)
            rd2_row = grp_pool.tile([1, NB], f32, tag="rd2_row")
            nc.vector.reciprocal(out=rd2_row, in_=d2_ps)
            rd2_ps = ps_g.tile([P, NB], f32, tag="mmb")
            nc.tensor.matmul(rd2_ps, ones_row, rd2_row, start=True, stop=True)
            ob3 = out_sb.rearrange("p (b r) -> p b r", r=R)
            nc.vector.tensor_mul(
                ob3[:, b0 : b0 + NB, :], ws3, bcast_inner(rd2_ps, R)
            )

        ngroup = B // NB
        chunks_per_group = (B // CB) // ngroup
        for gi in range(ngroup):
            for cc in range(chunks_per_group):
                phase_a_chunk(gi * chunks_per_group + cc)
            phase_b_group(gi)

        nc.sync.dma_start(
            out=out_ap.rearrange("b (p r) -> p b r", r=R),
            in_=out_sb.rearrange("p (b r) -> p b r", r=R),
        )


def _get_nc():
    if "nc" in _NC_CACHE:
        return _NC_CACHE["nc"]
    from concourse import bacc, mybir

    f32 = mybir.dt.float32
    nc = bacc.Bacc("TRN2", debug=False, num_devices=NCORES)
    ins = {
        "memory": nc.dram_tensor("memory", [B, N, M], f32, kind="ExternalInput").ap(),
        "k": nc.dram_tensor("k", [B, M], f32, kind="ExternalInput").ap(),
        "beta": nc.dram_tensor("beta", [B, 1], f32, kind="ExternalInput").ap(),
        "prev_w": nc.dram_tensor("prev_w", [B, N], f32, kind="ExternalInput").ap(),
        "g": nc.dram_tensor("g", [B, 1], f32, kind="ExternalInput").ap(),
        "s": nc.dram_tensor("s", [B, 3], f32, kind="ExternalInput").ap(),
        "gamma": nc.dram_tensor("gamma", [B, 1], f32, kind="ExternalInput").ap(),
    }
    out_ap = nc.dram_tensor("out", [B, N], f32, kind="ExternalOutput").ap()
    _build_body(nc, out_ap, ins)
    nc.finalize()
    _NC_CACHE["nc"] = nc
    return nc


def _shard_inputs(inputs):
    arrs = {
        name: np.ascontiguousarray(np.asarray(inputs[name], dtype=np.float32))
        for name in ("memory", "k", "beta", "prev_w", "g", "s", "gamma")
    }
    in_maps = []
    for c in range(NCORES):
        sl = slice(c * B, (c + 1) * B)
        in_maps.append({name: np.ascontiguousarray(a[sl]) for name, a in arrs.items()})
    return in_maps


def run(inputs, trace=False):
    from concourse.bass_utils import run_bass_kernel_spmd

    nc = _get_nc()
    in_maps = _shard_inputs(inputs)
    res = run_bass_kernel_spmd(
        nc, in_maps, core_ids=list(range(NCORES)), trace=trace,
        **({"trace_cores": [0]} if trace else {}),
    )
    out = np.concatenate([r["out"] for r in res.results], axis=0)
    return out, res


def kernel(**inputs):
    out, _ = run(inputs, trace=False)
    return out
